# revision 18
# baseline (speedup 1.0000x reference)
"""Trainium2 Bass kernel for the DependencyLearner embedding-lookup problem.

Computation (see reference):
  words' = where(pad_mask, 0, words)                       (B,L) int
  E[b,l,m]  = dot(W[words'[b,l]], V[words'[b,m]])          per-sentence energies
  E'[b,l,m] = E + vb[words'[b,m]]
  pos[b] = sum_l mvr[l] * (E'[l, hp[l]] + wb[words'[l]])
  nh[b,l] = argmax_m (E[l,m] + M[l,m])    M = gumbel noise + masking (host const)
  neg[b] = sum_l mvr[l] * (E'[l, nh[l]] + wb[words'[l]])

Sharding: data-parallel over batch. 8 cores x 8 sentences. V/W tables are
replicated per core (concatenated + vb/wb folded in as column 256); token
rows are gathered on-device with one batched indirect DMA per sentence pair.
"""

import numpy as np

import concourse.bass as bass
import concourse.mybir as mybir
import concourse.tile as tile
from concourse import bacc
from concourse import bass_utils

# Problem constants (hardcoded per task instructions)
VOCAB = 100000
COVOCAB = 100000
D = 256
B = 64
L = 128
N_CORES = 8
SENT_PER_CORE = B // N_CORES  # 8
# interleaved table row: V(256) | vb | pad(7) | W(256) | wb | pad(7)
ROW = 528
OFF_V = 0
OFF_VB = 256
OFF_W = 264
OFF_WB = 520
BIG = 1.0e30

# consts tile column layout
C_IOTA = 0            # [128] iota along free dim (row l = 0..127), f32
C_IDENT = 128         # [128] fp32 identity matrix for PE transposes
C_HP = 256            # [8]  masked positive heads per sentence (f32; -1 = masked)
C_MVR = 264           # [8]  score mask (1 - mask_incl_root) per sentence
C_ONES_COL = 272      # [1]  ones column
C_ONES_ROW = 273      # [128] row of ones in partition 0
C_TOTAL = 401

FP32 = mybir.dt.float32
I32 = mybir.dt.int32
U32 = mybir.dt.uint32


def build_nc():
    nc = bacc.Bacc("TRN2", target_bir_lowering=False, debug=False)

    table = nc.dram_tensor("table", [VOCAB, ROW], FP32, kind="ExternalInput")
    ids = nc.dram_tensor("ids", [L, SENT_PER_CORE], I32, kind="ExternalInput")
    m_in = nc.dram_tensor("m", [L, SENT_PER_CORE, L], FP32, kind="ExternalInput")
    consts = nc.dram_tensor("consts", [L, C_TOTAL], FP32, kind="ExternalInput")
    out = nc.dram_tensor("out", [2 * SENT_PER_CORE, 1], FP32, kind="ExternalOutput")

    with tile.TileContext(nc) as tc:
        with (
            tc.tile_pool(name="persist", bufs=1) as persist,
            tc.tile_pool(name="work", bufs=3) as work,
            tc.tile_pool(name="colvec", bufs=3) as colvec,
            tc.tile_pool(name="tp_psum", bufs=3, space="PSUM") as tp_psum,
            tc.tile_pool(name="e_psum", bufs=2, space="PSUM") as e_psum,
            tc.tile_pool(name="s_psum", bufs=1, space="PSUM") as s_psum,
        ):
            consts_sb = persist.tile([L, C_TOTAL], FP32, tag="consts")
            ids_sb = persist.tile([L, SENT_PER_CORE], I32, tag="ids")
            m_sb = persist.tile([L, SENT_PER_CORE, L], FP32, tag="m")
            gat = persist.tile([L, SENT_PER_CORE, ROW], FP32, tag="gat")
            s_all = persist.tile([L, 2 * SENT_PER_CORE], FP32, tag="s_all")

            nc.sync.dma_start(consts_sb[:], consts[:])
            nc.sync.dma_start(ids_sb[:], ids[:])
            nc.sync.dma_start(m_sb[:], m_in[:])

            ident = consts_sb[:, C_IDENT:C_IDENT + 128]
            iota = consts_sb[:, C_IOTA:C_IOTA + 128]

            # one indirect gather per sentence: each descriptor pulls the
            # interleaved V|W row (2112B) for one token ([P,1] offsets are the
            # only HW-supported form)
            for j in range(SENT_PER_CORE):
                nc.gpsimd.indirect_dma_start(
                    out=gat[:, j, :],
                    out_offset=None,
                    in_=table[:],
                    in_offset=bass.IndirectOffsetOnAxis(
                        ap=ids_sb[:, j:j + 1], axis=0
                    ),
                )

            # vb rows: one small PE transpose per sentence ([128,1] -> [1,128],
            # base partition must be 0 for the K=1 matmul below)
            vbt = []
            for j in range(SENT_PER_CORE):
                vb_ps = tp_psum.tile([1, L], FP32, tag="t_ps")
                nc.tensor.transpose(
                    out=vb_ps[:],
                    in_=gat[:, j, OFF_VB:OFF_VB + 1],
                    identity=ident,
                )
                vb_sb = persist.tile([1, L], FP32, tag=f"vbt{j}")
                nc.scalar.copy(out=vb_sb[:], in_=vb_ps[:])
                vbt.append(vb_sb)

            for j in range(SENT_PER_CORE):
                # transpose V/W rows into [d, l] layout
                t_sb = []
                for off in (OFF_V, OFF_V + 128, OFF_W, OFF_W + 128):
                    t_ps = tp_psum.tile([L, L], FP32, tag="t_ps")
                    nc.tensor.transpose(
                        out=t_ps[:],
                        in_=gat[:, j, off:off + 128],
                        identity=ident,
                    )
                    t = work.tile([L, L], FP32, tag="t_sb")
                    nc.scalar.copy(out=t[:], in_=t_ps[:])
                    t_sb.append(t)
                vwt0, vwt1, wwt0, wwt1 = t_sb

                # energies E[l,m] = sum_d W[l,d] V[m,d]  (PSUM accumulation)
                e_ps = e_psum.tile([L, L], FP32, tag="e_ps")
                nc.tensor.matmul(out=e_ps[:], lhsT=wwt0[:], rhs=vwt0[:],
                                 start=True, stop=False)
                nc.tensor.matmul(out=e_ps[:], lhsT=wwt1[:], rhs=vwt1[:],
                                 start=False, stop=True)

                # logits = E + M  -> argmax over free dim
                logits = work.tile([L, L], FP32, tag="logits")
                nc.vector.tensor_add(out=logits[:], in0=e_ps[:], in1=m_sb[:, j, :])
                mx8 = colvec.tile([L, 8], FP32, tag="mx8")
                nc.vector.max(out=mx8[:], in_=logits[:])
                ix8 = colvec.tile([L, 8], U32, tag="ix8")
                nc.vector.max_index(out=ix8[:], in_max=mx8[:], in_values=logits[:])
                nh_f = colvec.tile([L, 1], FP32, tag="nh_f")
                nc.vector.tensor_copy(out=nh_f[:], in_=ix8[:, 0:1])

                # E' = E + vb[m] (row-broadcast via K=1 matmul)
                nc.tensor.matmul(
                    out=e_ps[:],
                    lhsT=consts_sb[0:1, C_ONES_ROW:C_ONES_ROW + 128],
                    rhs=vbt[j][0:1, :],
                    start=False, stop=True, skip_group_check=True,
                )

                # score extraction: one fused stt per head set
                junk = work.tile([L, L], FP32, tag="junk")
                sp_raw = colvec.tile([L, 1], FP32, tag="sp_raw")
                nc.vector.scalar_tensor_tensor(
                    out=junk[:], in0=iota, scalar=consts_sb[:, C_HP + j:C_HP + j + 1],
                    in1=e_ps[:], op0=mybir.AluOpType.is_equal,
                    op1=mybir.AluOpType.mult, accum_out=sp_raw[:],
                )
                junk2 = work.tile([L, L], FP32, tag="junk2")
                sn_raw = colvec.tile([L, 1], FP32, tag="sn_raw")
                nc.vector.scalar_tensor_tensor(
                    out=junk2[:], in0=iota, scalar=nh_f[:],
                    in1=e_ps[:], op0=mybir.AluOpType.is_equal,
                    op1=mybir.AluOpType.mult, accum_out=sn_raw[:],
                )

                # s = (raw + wb) * mvr  -> columns of s_all
                wb_col = gat[:, j, OFF_WB:OFF_WB + 1]
                mvr_col = consts_sb[:, C_MVR + j:C_MVR + j + 1]
                nc.vector.scalar_tensor_tensor(
                    out=s_all[:, j:j + 1], in0=sp_raw[:], scalar=wb_col,
                    in1=mvr_col, op0=mybir.AluOpType.add, op1=mybir.AluOpType.mult,
                )
                nc.vector.scalar_tensor_tensor(
                    out=s_all[:, SENT_PER_CORE + j:SENT_PER_CORE + j + 1],
                    in0=sn_raw[:], scalar=wb_col,
                    in1=mvr_col, op0=mybir.AluOpType.add, op1=mybir.AluOpType.mult,
                )

            # reduce over positions: out[k] = sum_l s_all[l, k]
            s_ps = s_psum.tile([2 * SENT_PER_CORE, 1], FP32, tag="s_ps")
            nc.tensor.matmul(
                out=s_ps[:], lhsT=s_all[:],
                rhs=consts_sb[:, C_ONES_COL:C_ONES_COL + 1],
                start=True, stop=True,
            )
            s_out = persist.tile([2 * SENT_PER_CORE, 1], FP32, tag="s_out")
            nc.vector.tensor_copy(out=s_out[:], in_=s_ps[:])
            nc.sync.dma_start(out[:], s_out[:])

    nc.compile()
    return nc


_NC = None
TRACE = False
LAST_RES = None


def _get_nc():
    global _NC
    if _NC is None:
        _NC = build_nc()
    return _NC


def _gumbel_noise(words):
    """Gumbel noise exactly as jax.random.categorical(key(123), logits) draws it.

    jax.random bits differ between the cpu backend and the neuron/axon
    backend, so detect which backend generated the inputs (setup_inputs uses
    jax.random too) and draw the noise on that same backend.
    """
    import jax
    import jax.numpy as jnp

    def draw_words():
        ks = jax.random.split(jax.random.key(0), 8)
        return np.asarray(jax.device_get(
            jax.random.randint(ks[0], (B, L), 0, VOCAB, dtype=jnp.int32)))

    def draw_g():
        return np.asarray(jax.device_get(
            jax.random.gumbel(jax.random.key(123), (B, L, L), jnp.float32)))

    words = np.asarray(words)
    candidates = []
    try:
        cpu = jax.devices("cpu")[0]
        candidates.append(("cpu", cpu))
    except Exception:
        pass
    candidates.append(("default", None))

    for name, dev in candidates:
        try:
            if dev is not None:
                with jax.default_device(dev):
                    if np.array_equal(draw_words(), words):
                        return draw_g()
            else:
                if np.array_equal(draw_words(), words):
                    return draw_g()
        except Exception:
            continue
    # no backend reproduced the inputs; fall back to the default backend
    return draw_g()


def _prep_host(positive_sentences, mask, V, W, vb, wb):
    words = np.asarray(positive_sentences)[:, 0, :].astype(np.int64)
    heads = np.asarray(positive_sentences)[:, 1, :].astype(np.int64)
    pad = np.asarray(mask).astype(bool)

    wordsp = np.where(pad, 0, words)                       # (B, L)
    mir = pad.copy()
    mir[:, 0] = True                                       # mask incl root
    hp = np.where(mir, -1.0, heads).astype(np.float32)
    mvr = (~mir).astype(np.float32)                        # (B, L)

    # interleaved table: V(256) | vb | pad | W(256) | wb | pad per token
    tbl = np.zeros((VOCAB, ROW), dtype=np.float32)
    tbl[:, OFF_V:OFF_V + D] = np.asarray(V, dtype=np.float32)
    tbl[:, OFF_VB] = np.asarray(vb, dtype=np.float32)
    tbl[:, OFF_W:OFF_W + D] = np.asarray(W, dtype=np.float32)
    tbl[:, OFF_WB] = np.asarray(wb, dtype=np.float32)

    # logits additive term: gumbel noise + ok-masking
    G = _gumbel_noise(words)                               # (B, L, L)
    valid = ~pad
    eye = np.eye(L, dtype=bool)
    ok = valid[:, :, None] & valid[:, None, :] & ~eye[None, :, :]
    M = np.where(ok, G, G - BIG).astype(np.float32)        # (B, L, L)

    return wordsp, hp, mvr, tbl, M


def kernel(batch_id, positive_sentences, mask, V, W, vb, wb):
    wordsp, hp, mvr, tbl, M = _prep_host(positive_sentences, mask, V, W, vb, wb)

    in_maps = []
    for c in range(N_CORES):
        s0 = c * SENT_PER_CORE
        ids = np.zeros((L, SENT_PER_CORE), dtype=np.int32)
        consts = np.zeros((L, C_TOTAL), dtype=np.float32)
        consts[:, C_IOTA:C_IOTA + 128] = np.arange(L, dtype=np.float32)[None, :]
        consts[:, C_IDENT:C_IDENT + 128] = np.eye(L, dtype=np.float32)
        consts[:, C_ONES_COL] = 1.0
        consts[0, C_ONES_ROW:C_ONES_ROW + 128] = 1.0
        for j in range(SENT_PER_CORE):
            b = s0 + j
            ids[:, j] = wordsp[b]
            consts[:, C_HP + j] = hp[b]
            consts[:, C_MVR + j] = mvr[b]
        m_core = np.ascontiguousarray(
            np.transpose(M[s0:s0 + SENT_PER_CORE], (1, 0, 2))
        )  # (L, 8, L)
        in_maps.append({"table": tbl, "ids": ids, "m": m_core, "consts": consts})

    nc = _get_nc()
    res = bass_utils.run_bass_kernel_spmd(
        nc, in_maps, core_ids=list(range(N_CORES)), trace=TRACE
    )
    global LAST_RES
    LAST_RES = res

    pos = np.zeros(B, dtype=np.float32)
    neg = np.zeros(B, dtype=np.float32)
    for c in range(N_CORES):
        o = np.asarray(res.results[c]["out"]).reshape(2 * SENT_PER_CORE)
        pos[c * SENT_PER_CORE:(c + 1) * SENT_PER_CORE] = o[:SENT_PER_CORE]
        neg[c * SENT_PER_CORE:(c + 1) * SENT_PER_CORE] = o[SENT_PER_CORE:]
    return pos, neg


# revision 22
# speedup vs baseline: 1.5131x; 1.5131x over previous
"""Trainium2 Bass kernel for the DependencyLearner embedding-lookup problem.

Computation (see reference):
  words' = where(pad_mask, 0, words)                       (B,L) int
  E[b,l,m]  = dot(W[words'[b,l]], V[words'[b,m]])          per-sentence energies
  pos[b] = sum_l mvr[l] * (E[l, hp[l]] + vb[w'[hp[l]]] + wb[w'[l]])
  nh[b,l] = argmax_m (E[l,m] + M[l,m])    M = gumbel noise + masking (host-built)
  neg[b] = sum_l mvr[l] * (E[l, nh[l]] + vb[w'[nh[l]]] + wb[w'[l]])

Device-side trick for the neg score: ship M2 = M - vb_row_broadcast; then
E[l,nh] + vb[nh] = max_m(E+M) - M2[l,nh], so no vb row-broadcast matmul is
needed on device.

Sharding: data-parallel over batch. 8 cores x 8 sentences. The V/W tables are
replicated per core as one interleaved table (V|vb|W|wb per token row); token
rows are gathered on-device with one [P,1]-offset indirect DMA per sentence.
"""

import numpy as np

import concourse.bass as bass
import concourse.mybir as mybir
import concourse.tile as tile
from concourse import bacc
from concourse import bass_utils

# Problem constants (hardcoded per task instructions)
VOCAB = 100000
COVOCAB = 100000
D = 256
B = 64
L = 128
N_CORES = 8
SENT_PER_CORE = B // N_CORES  # 8
# interleaved table row: V(256) | vb | pad(7) | W(256) | wb | pad(7)
ROW = 528
OFF_V = 0
OFF_VB = 256
OFF_W = 264
OFF_WB = 520
BIG = 1.0e30

# consts tile column layout (fp32)
C_IOTA = 0            # [128] iota along free dim (row l = 0..127)
C_HP = 128            # [8]  masked positive heads per sentence (-1 = masked)
C_MVR = 136           # [8]  score mask (1 - mask_incl_root) per sentence
C_VBHP = 144          # [8]  vb[words'[hp]] per sentence (0 where masked)
C_ONES_COL = 152      # [1]  ones column
C_TOTAL = 153

FP32 = mybir.dt.float32
BF16 = mybir.dt.bfloat16
I32 = mybir.dt.int32
U32 = mybir.dt.uint32


def build_nc():
    nc = bacc.Bacc("TRN2", target_bir_lowering=False, debug=False)

    table = nc.dram_tensor("table", [VOCAB, ROW], FP32, kind="ExternalInput")
    ids = nc.dram_tensor("ids", [L, SENT_PER_CORE], I32, kind="ExternalInput")
    m_in = nc.dram_tensor("m", [L, SENT_PER_CORE, L], FP32, kind="ExternalInput")
    m2_in = nc.dram_tensor("m2", [L, SENT_PER_CORE, L], FP32, kind="ExternalInput")
    consts = nc.dram_tensor("consts", [L, C_TOTAL], FP32, kind="ExternalInput")
    identb = nc.dram_tensor("identb", [L, L], FP32, kind="ExternalInput")
    out = nc.dram_tensor("out", [2 * SENT_PER_CORE, 1], FP32, kind="ExternalOutput")

    NS = SENT_PER_CORE

    with tile.TileContext(nc) as tc:
        with (
            tc.tile_pool(name="persist", bufs=1) as persist,
            tc.tile_pool(name="work", bufs=3) as work,
            tc.tile_pool(name="colvec", bufs=2) as colvec,
            tc.tile_pool(name="tq_psum", bufs=3, space="PSUM") as tq_psum,
            tc.tile_pool(name="e_psum", bufs=2, space="PSUM") as e_psum,
            tc.tile_pool(name="s_psum", bufs=1, space="PSUM") as s_psum,
        ):
            ids_sb = persist.tile([L, NS], I32, tag="ids")
            consts_sb = persist.tile([L, C_TOTAL], FP32, tag="consts")
            ident_sb = persist.tile([L, L], FP32, tag="identb")
            m_sb = persist.tile([L, NS, L], FP32, tag="m")
            m2_sb = persist.tile([L, NS, L], FP32, tag="m2")
            gat = persist.tile([L, NS, ROW], FP32, tag="gat")
            s_all = persist.tile([L, 2 * NS], FP32, tag="s_all")
            mx64 = persist.tile([L, 8 * NS], FP32, tag="mx64")
            ix64 = persist.tile([L, 8 * NS], U32, tag="ix64")
            nh8 = persist.tile([L, NS], FP32, tag="nh8")
            spraw8 = persist.tile([L, NS], FP32, tag="spraw8")
            m2at8 = persist.tile([L, NS], FP32, tag="m2at8")

            # ids first: the gathers depend only on it
            nc.sync.dma_start(ids_sb[:], ids[:])
            nc.sync.dma_start(consts_sb[:], consts[:])
            nc.sync.dma_start(ident_sb[:], identb[:])

            # one indirect gather per sentence: each descriptor pulls the
            # interleaved V|W row (2112B) for one token ([P,1] offsets are the
            # only HW-supported form)
            for j in range(NS):
                nc.gpsimd.indirect_dma_start(
                    out=gat[:, j, :],
                    out_offset=None,
                    in_=table[:],
                    in_offset=bass.IndirectOffsetOnAxis(
                        ap=ids_sb[:, j:j + 1], axis=0
                    ),
                )

            nc.sync.dma_start(m_sb[:], m_in[:])
            nc.sync.dma_start(m2_sb[:], m2_in[:])

            iota = consts_sb[:, C_IOTA:C_IOTA + 128]

            # per sentence: 4 PE transposes into one PSUM bank, 1 ACT copy out
            tq_sb = []
            for j in range(NS):
                tq = tq_psum.tile([L, 512], FP32, tag="tq_ps")
                for k, off in enumerate((OFF_V, OFF_V + 128, OFF_W, OFF_W + 128)):
                    nc.tensor.matmul(
                        out=tq[:, k * 128:(k + 1) * 128],
                        lhsT=gat[:, j, off:off + 128],
                        rhs=ident_sb[:],
                        is_transpose=True,
                        start=(k == 0), stop=(k == 3),
                    )
                t = work.tile([L, 512], FP32, tag="tq_sb")
                nc.scalar.copy(out=t[:], in_=tq[:])
                tq_sb.append(t)

            # energies: sentence pairs share one PSUM bank [128, 256]
            for g in range(NS // 2):
                e2 = e_psum.tile([L, 256], FP32, tag="e2")
                for jj in range(2):
                    t = tq_sb[2 * g + jj]
                    nc.tensor.matmul(out=e2[:, jj * 128:(jj + 1) * 128],
                                     lhsT=t[:, 256:384], rhs=t[:, 0:128],
                                     start=(jj == 0), stop=False)
                    nc.tensor.matmul(out=e2[:, jj * 128:(jj + 1) * 128],
                                     lhsT=t[:, 384:512], rhs=t[:, 128:256],
                                     start=False, stop=(jj == 1))

                # logits for the pair in one op
                logits = work.tile([L, 256], FP32, tag="logits")
                nc.vector.tensor_add(out=logits[:], in0=e2[:],
                                     in1=m_sb[:, 2 * g:2 * g + 2, :])

                for jj in range(2):
                    j = 2 * g + jj
                    lj = logits[:, jj * 128:(jj + 1) * 128]
                    nc.vector.max(out=mx64[:, 8 * j:8 * j + 8], in_=lj)
                    nc.vector.max_index(out=ix64[:, 8 * j:8 * j + 8],
                                        in_max=mx64[:, 8 * j:8 * j + 8],
                                        in_values=lj)
                    # pos score: E[l, hp[l]] extracted from PSUM
                    junk = work.tile([L, L], FP32, tag="junk")
                    nc.vector.scalar_tensor_tensor(
                        out=junk[:], in0=iota,
                        scalar=consts_sb[:, C_HP + j:C_HP + j + 1],
                        in1=e2[:, jj * 128:(jj + 1) * 128],
                        op0=mybir.AluOpType.is_equal,
                        op1=mybir.AluOpType.mult,
                        accum_out=spraw8[:, j:j + 1],
                    )

            # neg-head indices -> f32 in one strided cast
            nc.vector.tensor_copy(out=nh8[:], in_=ix64[:, 0::8])

            # neg extraction: M2[l, nh[l]] from SBUF
            for j in range(NS):
                junk2 = work.tile([L, L], FP32, tag="junk2")
                nc.vector.scalar_tensor_tensor(
                    out=junk2[:], in0=iota, scalar=nh8[:, j:j + 1],
                    in1=m2_sb[:, j, :],
                    op0=mybir.AluOpType.is_equal, op1=mybir.AluOpType.mult,
                    accum_out=m2at8[:, j:j + 1],
                )

            # batched finalize
            wb8 = gat[:, :, OFF_WB]                      # [128, 8] strided
            mvr8 = consts_sb[:, C_MVR:C_MVR + NS]
            vbhp8 = consts_sb[:, C_VBHP:C_VBHP + NS]
            t1 = persist.tile([L, NS], FP32, tag="t1")
            nc.vector.tensor_add(out=t1[:], in0=spraw8[:], in1=vbhp8)
            t2 = persist.tile([L, NS], FP32, tag="t2")
            nc.vector.tensor_add(out=t2[:], in0=t1[:], in1=wb8)
            nc.vector.tensor_mul(out=s_all[:, 0:NS], in0=t2[:], in1=mvr8)

            t3 = persist.tile([L, NS], FP32, tag="t3")
            nc.vector.tensor_tensor(out=t3[:], in0=mx64[:, 0::8], in1=m2at8[:],
                                    op=mybir.AluOpType.subtract)
            t4 = persist.tile([L, NS], FP32, tag="t4")
            nc.vector.tensor_add(out=t4[:], in0=t3[:], in1=wb8)
            nc.vector.tensor_mul(out=s_all[:, NS:2 * NS], in0=t4[:], in1=mvr8)

            # reduce over positions: out[k] = sum_l s_all[l, k]
            s_ps = s_psum.tile([2 * NS, 1], FP32, tag="s_ps")
            nc.tensor.matmul(
                out=s_ps[:], lhsT=s_all[:],
                rhs=consts_sb[:, C_ONES_COL:C_ONES_COL + 1],
                start=True, stop=True,
            )
            s_out = persist.tile([2 * NS, 1], FP32, tag="s_out")
            nc.vector.tensor_copy(out=s_out[:], in_=s_ps[:])
            nc.sync.dma_start(out[:], s_out[:])

    nc.compile()
    return nc


_NC = None
TRACE = False
LAST_RES = None


def _get_nc():
    global _NC
    if _NC is None:
        _NC = build_nc()
    return _NC


def _gumbel_noise(words):
    """Gumbel noise exactly as jax.random.categorical(key(123), logits) draws it.

    jax.random bits differ between the cpu backend and the neuron/axon
    backend, so detect which backend generated the inputs (setup_inputs uses
    jax.random too) and draw the noise on that same backend.
    """
    import jax
    import jax.numpy as jnp

    def draw_words():
        ks = jax.random.split(jax.random.key(0), 8)
        return np.asarray(jax.device_get(
            jax.random.randint(ks[0], (B, L), 0, VOCAB, dtype=jnp.int32)))

    def draw_g():
        return np.asarray(jax.device_get(
            jax.random.gumbel(jax.random.key(123), (B, L, L), jnp.float32)))

    words = np.asarray(words)
    candidates = []
    try:
        cpu = jax.devices("cpu")[0]
        candidates.append(("cpu", cpu))
    except Exception:
        pass
    candidates.append(("default", None))

    for name, dev in candidates:
        try:
            if dev is not None:
                with jax.default_device(dev):
                    if np.array_equal(draw_words(), words):
                        return draw_g()
            else:
                if np.array_equal(draw_words(), words):
                    return draw_g()
        except Exception:
            continue
    # no backend reproduced the inputs; fall back to the default backend
    return draw_g()


def _prep_host(positive_sentences, mask, V, W, vb, wb):
    words = np.asarray(positive_sentences)[:, 0, :].astype(np.int64)
    heads = np.asarray(positive_sentences)[:, 1, :].astype(np.int64)
    pad = np.asarray(mask).astype(bool)
    vb = np.asarray(vb, dtype=np.float32)

    wordsp = np.where(pad, 0, words)                       # (B, L)
    mir = pad.copy()
    mir[:, 0] = True                                       # mask incl root
    hp = np.where(mir, -1.0, heads).astype(np.float32)
    mvr = (~mir).astype(np.float32)                        # (B, L)
    # vb at the positive heads (host-known indices)
    vbhp = np.where(mir, 0.0, vb[np.take_along_axis(wordsp, heads, axis=1)])
    vbhp = vbhp.astype(np.float32)

    # interleaved table: V(256) | vb | pad | W(256) | wb | pad per token
    tbl = np.zeros((VOCAB, ROW), dtype=np.float32)
    tbl[:, OFF_V:OFF_V + D] = np.asarray(V, dtype=np.float32)
    tbl[:, OFF_VB] = vb
    tbl[:, OFF_W:OFF_W + D] = np.asarray(W, dtype=np.float32)
    tbl[:, OFF_WB] = np.asarray(wb, dtype=np.float32)

    # logits additive term: gumbel noise + ok-masking; M2 = M - vb[words'[m]]
    G = _gumbel_noise(words)                               # (B, L, L)
    valid = ~pad
    eye = np.eye(L, dtype=bool)
    ok = valid[:, :, None] & valid[:, None, :] & ~eye[None, :, :]
    M = np.where(ok, G, G - BIG).astype(np.float32)        # (B, L, L)
    M2 = (M - vb[wordsp][:, None, :]).astype(np.float32)

    return wordsp, hp, mvr, vbhp, tbl, M, M2


def _build_in_maps(positive_sentences, mask, V, W, vb, wb):
    wordsp, hp, mvr, vbhp, tbl, M, M2 = _prep_host(
        positive_sentences, mask, V, W, vb, wb)

    identb = np.eye(L, dtype=np.float32)

    in_maps = []
    for c in range(N_CORES):
        s0 = c * SENT_PER_CORE
        ids = np.zeros((L, SENT_PER_CORE), dtype=np.int32)
        consts = np.zeros((L, C_TOTAL), dtype=np.float32)
        consts[:, C_IOTA:C_IOTA + 128] = np.arange(L, dtype=np.float32)[None, :]
        consts[:, C_ONES_COL] = 1.0
        for j in range(SENT_PER_CORE):
            b = s0 + j
            ids[:, j] = wordsp[b]
            consts[:, C_HP + j] = hp[b]
            consts[:, C_MVR + j] = mvr[b]
            consts[:, C_VBHP + j] = vbhp[b]
        m_core = np.ascontiguousarray(
            np.transpose(M[s0:s0 + SENT_PER_CORE], (1, 0, 2)))
        m2_core = np.ascontiguousarray(
            np.transpose(M2[s0:s0 + SENT_PER_CORE], (1, 0, 2)))
        in_maps.append({"table": tbl, "ids": ids, "m": m_core, "m2": m2_core,
                        "consts": consts, "identb": identb})
    return in_maps


def kernel(batch_id, positive_sentences, mask, V, W, vb, wb):
    in_maps = _build_in_maps(positive_sentences, mask, V, W, vb, wb)
    nc = _get_nc()
    res = bass_utils.run_bass_kernel_spmd(
        nc, in_maps, core_ids=list(range(N_CORES)), trace=TRACE
    )
    global LAST_RES
    LAST_RES = res

    pos = np.zeros(B, dtype=np.float32)
    neg = np.zeros(B, dtype=np.float32)
    for c in range(N_CORES):
        o = np.asarray(res.results[c]["out"]).reshape(2 * SENT_PER_CORE)
        pos[c * SENT_PER_CORE:(c + 1) * SENT_PER_CORE] = o[:SENT_PER_CORE]
        neg[c * SENT_PER_CORE:(c + 1) * SENT_PER_CORE] = o[SENT_PER_CORE:]
    return pos, neg


# revision 24
# speedup vs baseline: 1.5277x; 1.0097x over previous
"""Trainium2 Bass kernel for the DependencyLearner embedding-lookup problem.

Computation (see reference):
  words' = where(pad_mask, 0, words)                       (B,L) int
  E[b,l,m]  = dot(W[words'[b,l]], V[words'[b,m]])          per-sentence energies
  pos[b] = sum_l mvr[l] * (E[l, hp[l]] + vb[w'[hp[l]]] + wb[w'[l]])
  nh[b,l] = argmax_m (E[l,m] + M[l,m])    M = gumbel noise + masking (host-built)
  neg[b] = sum_l mvr[l] * (E[l, nh[l]] + vb[w'[nh[l]]] + wb[w'[l]])

Device-side trick for the neg score: ship M2 = M - vb_row_broadcast; then
E[l,nh] + vb[nh] = max_m(E+M) - M2[l,nh], so no vb row-broadcast matmul is
needed on device.

Sharding: data-parallel over batch. 8 cores x 8 sentences. The V/W tables are
replicated per core as one interleaved table (V|vb|W|wb per token row); token
rows are gathered on-device with one [P,1]-offset indirect DMA per sentence.
"""

import numpy as np

import concourse.bass as bass
import concourse.mybir as mybir
import concourse.tile as tile
from concourse import bacc
from concourse import bass_utils

# Problem constants (hardcoded per task instructions)
VOCAB = 100000
COVOCAB = 100000
D = 256
B = 64
L = 128
N_CORES = 8
SENT_PER_CORE = B // N_CORES  # 8
# interleaved table row: V(256) | vb | pad(7) | W(256) | wb | pad(7)
ROW = 528
OFF_V = 0
OFF_VB = 256
OFF_W = 264
OFF_WB = 520
BIG = 1.0e30

# consts tile column layout (fp32)
C_IOTA = 0            # [128] iota along free dim (row l = 0..127)
C_HP = 128            # [8]  masked positive heads per sentence (-1 = masked)
C_MVR = 136           # [8]  score mask (1 - mask_incl_root) per sentence
C_VBHP = 144          # [8]  vb[words'[hp]] per sentence (0 where masked)
C_ONES_COL = 152      # [1]  ones column
C_TOTAL = 153

FP32 = mybir.dt.float32
BF16 = mybir.dt.bfloat16
I32 = mybir.dt.int32
U32 = mybir.dt.uint32


def build_nc():
    nc = bacc.Bacc("TRN2", target_bir_lowering=False, debug=False)

    table = nc.dram_tensor("table", [VOCAB, ROW], FP32, kind="ExternalInput")
    ids = nc.dram_tensor("ids", [L, SENT_PER_CORE], I32, kind="ExternalInput")
    m_in = nc.dram_tensor("m", [L, SENT_PER_CORE, L], FP32, kind="ExternalInput")
    m2_in = nc.dram_tensor("m2", [L, SENT_PER_CORE, L], FP32, kind="ExternalInput")
    consts = nc.dram_tensor("consts", [L, C_TOTAL], FP32, kind="ExternalInput")
    identb = nc.dram_tensor("identb", [L, L], FP32, kind="ExternalInput")
    out = nc.dram_tensor("out", [2 * SENT_PER_CORE, 1], FP32, kind="ExternalOutput")

    NS = SENT_PER_CORE

    with tile.TileContext(nc) as tc:
        with (
            tc.tile_pool(name="persist", bufs=1) as persist,
            tc.tile_pool(name="work", bufs=3) as work,
            tc.tile_pool(name="colvec", bufs=2) as colvec,
            tc.tile_pool(name="tq_psum", bufs=3, space="PSUM") as tq_psum,
            tc.tile_pool(name="e_psum", bufs=2, space="PSUM") as e_psum,
            tc.tile_pool(name="s_psum", bufs=1, space="PSUM") as s_psum,
        ):
            ids_sb = persist.tile([L, NS], I32, tag="ids")
            consts_sb = persist.tile([L, C_TOTAL], FP32, tag="consts")
            ident_sb = persist.tile([L, L], FP32, tag="identb")
            m_sb = persist.tile([L, NS, L], FP32, tag="m")
            m2_sb = persist.tile([L, NS, L], FP32, tag="m2")
            gat = persist.tile([L, NS, ROW], FP32, tag="gat")
            s_all = persist.tile([L, 2 * NS], FP32, tag="s_all")
            mx64 = persist.tile([L, 8 * NS], FP32, tag="mx64")
            ix64 = persist.tile([L, 8 * NS], U32, tag="ix64")
            nh8 = persist.tile([L, NS], FP32, tag="nh8")
            spraw8 = persist.tile([L, NS], FP32, tag="spraw8")
            m2at8 = persist.tile([L, NS], FP32, tag="m2at8")

            # ids first: the gathers depend only on it; m/m2 next so their
            # transfers finish before the gather stream needs the bandwidth
            nc.sync.dma_start(ids_sb[:], ids[:])
            nc.sync.dma_start(m_sb[:], m_in[:])
            nc.sync.dma_start(m2_sb[:], m2_in[:])
            nc.sync.dma_start(consts_sb[:], consts[:])
            nc.sync.dma_start(ident_sb[:], identb[:])

            # one indirect gather per sentence: each descriptor pulls the
            # interleaved V|W row (2112B) for one token ([P,1] offsets are the
            # only HW-supported form)
            for j in range(NS):
                nc.gpsimd.indirect_dma_start(
                    out=gat[:, j, :],
                    out_offset=None,
                    in_=table[:],
                    in_offset=bass.IndirectOffsetOnAxis(
                        ap=ids_sb[:, j:j + 1], axis=0
                    ),
                )

            iota = consts_sb[:, C_IOTA:C_IOTA + 128]

            # per sentence: 4 PE transposes into one PSUM bank laid out
            # [Vc0|Wc0|Vc1|Wc1], then 2 half ACT copies so the chunk-0 energy
            # matmul can start while chunk 1 still transposes
            tq_sb = []
            for j in range(NS):
                tq = tq_psum.tile([L, 512], FP32, tag="tq_ps")
                t = work.tile([L, 512], FP32, tag="tq_sb")
                for k, off in enumerate((OFF_V, OFF_W, OFF_V + 128, OFF_W + 128)):
                    nc.tensor.matmul(
                        out=tq[:, k * 128:(k + 1) * 128],
                        lhsT=gat[:, j, off:off + 128],
                        rhs=ident_sb[:],
                        is_transpose=True,
                        start=(k == 0), stop=(k == 1),
                        skip_group_check=(k >= 2),
                    )
                    if k == 1:
                        nc.scalar.copy(out=t[:, 0:256], in_=tq[:, 0:256])
                nc.scalar.copy(out=t[:, 256:512], in_=tq[:, 256:512])
                tq_sb.append(t)

            # energies: sentence pairs share one PSUM bank [128, 256]
            for g in range(NS // 2):
                e2 = e_psum.tile([L, 256], FP32, tag="e2")
                for jj in range(2):
                    t = tq_sb[2 * g + jj]
                    nc.tensor.matmul(out=e2[:, jj * 128:(jj + 1) * 128],
                                     lhsT=t[:, 128:256], rhs=t[:, 0:128],
                                     start=(jj == 0), stop=False)
                    nc.tensor.matmul(out=e2[:, jj * 128:(jj + 1) * 128],
                                     lhsT=t[:, 384:512], rhs=t[:, 256:384],
                                     start=False, stop=(jj == 1))

                # logits for the pair in one op
                logits = work.tile([L, 256], FP32, tag="logits")
                nc.vector.tensor_add(out=logits[:], in0=e2[:],
                                     in1=m_sb[:, 2 * g:2 * g + 2, :])

                for jj in range(2):
                    j = 2 * g + jj
                    lj = logits[:, jj * 128:(jj + 1) * 128]
                    nc.vector.max(out=mx64[:, 8 * j:8 * j + 8], in_=lj)
                    nc.vector.max_index(out=ix64[:, 8 * j:8 * j + 8],
                                        in_max=mx64[:, 8 * j:8 * j + 8],
                                        in_values=lj)
                    # pos score: E[l, hp[l]] extracted from PSUM
                    junk = work.tile([L, L], FP32, tag="junk")
                    nc.vector.scalar_tensor_tensor(
                        out=junk[:], in0=iota,
                        scalar=consts_sb[:, C_HP + j:C_HP + j + 1],
                        in1=e2[:, jj * 128:(jj + 1) * 128],
                        op0=mybir.AluOpType.is_equal,
                        op1=mybir.AluOpType.mult,
                        accum_out=spraw8[:, j:j + 1],
                    )
                # per-pair: neg-head indices -> f32, then extract M2[l, nh[l]]
                nc.vector.tensor_copy(out=nh8[:, 2 * g:2 * g + 2],
                                      in_=ix64[:, 16 * g:16 * g + 16:8])
                for jj in range(2):
                    j = 2 * g + jj
                    junk2 = work.tile([L, L], FP32, tag="junk2")
                    nc.vector.scalar_tensor_tensor(
                        out=junk2[:], in0=iota, scalar=nh8[:, j:j + 1],
                        in1=m2_sb[:, j, :],
                        op0=mybir.AluOpType.is_equal, op1=mybir.AluOpType.mult,
                        accum_out=m2at8[:, j:j + 1],
                    )

            # batched finalize
            wb8 = gat[:, :, OFF_WB]                      # [128, 8] strided
            mvr8 = consts_sb[:, C_MVR:C_MVR + NS]
            vbhp8 = consts_sb[:, C_VBHP:C_VBHP + NS]
            t1 = persist.tile([L, NS], FP32, tag="t1")
            nc.vector.tensor_add(out=t1[:], in0=spraw8[:], in1=vbhp8)
            t2 = persist.tile([L, NS], FP32, tag="t2")
            nc.vector.tensor_add(out=t2[:], in0=t1[:], in1=wb8)
            nc.vector.tensor_mul(out=s_all[:, 0:NS], in0=t2[:], in1=mvr8)

            t3 = persist.tile([L, NS], FP32, tag="t3")
            nc.vector.tensor_tensor(out=t3[:], in0=mx64[:, 0::8], in1=m2at8[:],
                                    op=mybir.AluOpType.subtract)
            t4 = persist.tile([L, NS], FP32, tag="t4")
            nc.vector.tensor_add(out=t4[:], in0=t3[:], in1=wb8)
            nc.vector.tensor_mul(out=s_all[:, NS:2 * NS], in0=t4[:], in1=mvr8)

            # reduce over positions: out[k] = sum_l s_all[l, k]
            s_ps = s_psum.tile([2 * NS, 1], FP32, tag="s_ps")
            nc.tensor.matmul(
                out=s_ps[:], lhsT=s_all[:],
                rhs=consts_sb[:, C_ONES_COL:C_ONES_COL + 1],
                start=True, stop=True,
            )
            s_out = persist.tile([2 * NS, 1], FP32, tag="s_out")
            nc.vector.tensor_copy(out=s_out[:], in_=s_ps[:])
            nc.sync.dma_start(out[:], s_out[:])

    nc.compile()
    return nc


_NC = None
TRACE = False
LAST_RES = None


def _get_nc():
    global _NC
    if _NC is None:
        _NC = build_nc()
    return _NC


def _gumbel_noise(words):
    """Gumbel noise exactly as jax.random.categorical(key(123), logits) draws it.

    jax.random bits differ between the cpu backend and the neuron/axon
    backend, so detect which backend generated the inputs (setup_inputs uses
    jax.random too) and draw the noise on that same backend.
    """
    import jax
    import jax.numpy as jnp

    def draw_words():
        ks = jax.random.split(jax.random.key(0), 8)
        return np.asarray(jax.device_get(
            jax.random.randint(ks[0], (B, L), 0, VOCAB, dtype=jnp.int32)))

    def draw_g():
        return np.asarray(jax.device_get(
            jax.random.gumbel(jax.random.key(123), (B, L, L), jnp.float32)))

    words = np.asarray(words)
    candidates = []
    try:
        cpu = jax.devices("cpu")[0]
        candidates.append(("cpu", cpu))
    except Exception:
        pass
    candidates.append(("default", None))

    for name, dev in candidates:
        try:
            if dev is not None:
                with jax.default_device(dev):
                    if np.array_equal(draw_words(), words):
                        return draw_g()
            else:
                if np.array_equal(draw_words(), words):
                    return draw_g()
        except Exception:
            continue
    # no backend reproduced the inputs; fall back to the default backend
    return draw_g()


def _prep_host(positive_sentences, mask, V, W, vb, wb):
    words = np.asarray(positive_sentences)[:, 0, :].astype(np.int64)
    heads = np.asarray(positive_sentences)[:, 1, :].astype(np.int64)
    pad = np.asarray(mask).astype(bool)
    vb = np.asarray(vb, dtype=np.float32)

    wordsp = np.where(pad, 0, words)                       # (B, L)
    mir = pad.copy()
    mir[:, 0] = True                                       # mask incl root
    hp = np.where(mir, -1.0, heads).astype(np.float32)
    mvr = (~mir).astype(np.float32)                        # (B, L)
    # vb at the positive heads (host-known indices)
    vbhp = np.where(mir, 0.0, vb[np.take_along_axis(wordsp, heads, axis=1)])
    vbhp = vbhp.astype(np.float32)

    # interleaved table: V(256) | vb | pad | W(256) | wb | pad per token
    tbl = np.zeros((VOCAB, ROW), dtype=np.float32)
    tbl[:, OFF_V:OFF_V + D] = np.asarray(V, dtype=np.float32)
    tbl[:, OFF_VB] = vb
    tbl[:, OFF_W:OFF_W + D] = np.asarray(W, dtype=np.float32)
    tbl[:, OFF_WB] = np.asarray(wb, dtype=np.float32)

    # logits additive term: gumbel noise + ok-masking; M2 = M - vb[words'[m]]
    G = _gumbel_noise(words)                               # (B, L, L)
    valid = ~pad
    eye = np.eye(L, dtype=bool)
    ok = valid[:, :, None] & valid[:, None, :] & ~eye[None, :, :]
    M = np.where(ok, G, G - BIG).astype(np.float32)        # (B, L, L)
    M2 = (M - vb[wordsp][:, None, :]).astype(np.float32)

    return wordsp, hp, mvr, vbhp, tbl, M, M2


def _build_in_maps(positive_sentences, mask, V, W, vb, wb):
    wordsp, hp, mvr, vbhp, tbl, M, M2 = _prep_host(
        positive_sentences, mask, V, W, vb, wb)

    identb = np.eye(L, dtype=np.float32)

    in_maps = []
    for c in range(N_CORES):
        s0 = c * SENT_PER_CORE
        ids = np.zeros((L, SENT_PER_CORE), dtype=np.int32)
        consts = np.zeros((L, C_TOTAL), dtype=np.float32)
        consts[:, C_IOTA:C_IOTA + 128] = np.arange(L, dtype=np.float32)[None, :]
        consts[:, C_ONES_COL] = 1.0
        for j in range(SENT_PER_CORE):
            b = s0 + j
            ids[:, j] = wordsp[b]
            consts[:, C_HP + j] = hp[b]
            consts[:, C_MVR + j] = mvr[b]
            consts[:, C_VBHP + j] = vbhp[b]
        m_core = np.ascontiguousarray(
            np.transpose(M[s0:s0 + SENT_PER_CORE], (1, 0, 2)))
        m2_core = np.ascontiguousarray(
            np.transpose(M2[s0:s0 + SENT_PER_CORE], (1, 0, 2)))
        in_maps.append({"table": tbl, "ids": ids, "m": m_core, "m2": m2_core,
                        "consts": consts, "identb": identb})
    return in_maps


def kernel(batch_id, positive_sentences, mask, V, W, vb, wb):
    in_maps = _build_in_maps(positive_sentences, mask, V, W, vb, wb)
    nc = _get_nc()
    res = bass_utils.run_bass_kernel_spmd(
        nc, in_maps, core_ids=list(range(N_CORES)), trace=TRACE
    )
    global LAST_RES
    LAST_RES = res

    pos = np.zeros(B, dtype=np.float32)
    neg = np.zeros(B, dtype=np.float32)
    for c in range(N_CORES):
        o = np.asarray(res.results[c]["out"]).reshape(2 * SENT_PER_CORE)
        pos[c * SENT_PER_CORE:(c + 1) * SENT_PER_CORE] = o[:SENT_PER_CORE]
        neg[c * SENT_PER_CORE:(c + 1) * SENT_PER_CORE] = o[SENT_PER_CORE:]
    return pos, neg


# revision 26
# speedup vs baseline: 1.6239x; 1.0629x over previous
"""Trainium2 Bass kernel for the DependencyLearner embedding-lookup problem.

Computation (see reference):
  words' = where(pad_mask, 0, words)                       (B,L) int
  E[b,l,m]  = dot(W[words'[b,l]], V[words'[b,m]])          per-sentence energies
  pos[b] = sum_l mvr[l] * (E[l, hp[l]] + vb[w'[hp[l]]] + wb[w'[l]])
  nh[b,l] = argmax_m (E[l,m] + M[l,m])    M = gumbel noise + masking (host-built)
  neg[b] = sum_l mvr[l] * (E[l, nh[l]] + vb[w'[nh[l]]] + wb[w'[l]])

Device-side trick for the neg score: ship M2 = M - vb_row_broadcast; then
E[l,nh] + vb[nh] = max_m(E+M) - M2[l,nh], so no vb row-broadcast matmul is
needed on device.

Sharding: data-parallel over batch. 8 cores x 8 sentences. The V/W tables are
replicated per core as one interleaved table (V|vb|W|wb per token row); token
rows are gathered on-device with one [P,1]-offset indirect DMA per sentence.
"""

import numpy as np

import concourse.bass as bass
import concourse.mybir as mybir
import concourse.tile as tile
from concourse import bacc
from concourse import bass_utils

# Problem constants (hardcoded per task instructions)
VOCAB = 100000
COVOCAB = 100000
D = 256
B = 64
L = 128
N_CORES = 8
SENT_PER_CORE = B // N_CORES  # 8
# interleaved table row: V(256) | vb | pad(7) | W(256) | wb | pad(7)
ROW = 528
OFF_V = 0
OFF_VB = 256
OFF_W = 264
OFF_WB = 520
BIG = 1.0e30

# consts tile column layout (fp32)
C_IOTA = 0            # [128] iota along free dim (row l = 0..127)
C_HP = 128            # [8]  masked positive heads per sentence (-1 = masked)
C_MVR = 136           # [8]  score mask (1 - mask_incl_root) per sentence
C_VBHP = 144          # [8]  vb[words'[hp]] per sentence (0 where masked)
C_ONES_COL = 152      # [1]  ones column
C_TOTAL = 153

FP32 = mybir.dt.float32
BF16 = mybir.dt.bfloat16
I32 = mybir.dt.int32
U32 = mybir.dt.uint32


def build_nc():
    nc = bacc.Bacc("TRN2", target_bir_lowering=False, debug=False)

    table = nc.dram_tensor("table", [VOCAB, ROW], FP32, kind="ExternalInput")
    ids = nc.dram_tensor("ids", [L, SENT_PER_CORE], I32, kind="ExternalInput")
    m_in = nc.dram_tensor("m", [L, SENT_PER_CORE, L], FP32, kind="ExternalInput")
    m2_in = nc.dram_tensor("m2", [L, SENT_PER_CORE, L], FP32, kind="ExternalInput")
    consts = nc.dram_tensor("consts", [L, C_TOTAL], FP32, kind="ExternalInput")
    identb = nc.dram_tensor("identb", [L, L], FP32, kind="ExternalInput")
    out = nc.dram_tensor("out", [2 * SENT_PER_CORE, 1], FP32, kind="ExternalOutput")

    NS = SENT_PER_CORE

    with tile.TileContext(nc) as tc:
        with (
            tc.tile_pool(name="persist", bufs=1) as persist,
            tc.tile_pool(name="work", bufs=3) as work,
            tc.tile_pool(name="colvec", bufs=2) as colvec,
            tc.tile_pool(name="tq_psum", bufs=3, space="PSUM") as tq_psum,
            tc.tile_pool(name="e_psum", bufs=2, space="PSUM") as e_psum,
            tc.tile_pool(name="s_psum", bufs=1, space="PSUM") as s_psum,
        ):
            ids_sb = persist.tile([L, NS], I32, tag="ids")
            consts_sb = persist.tile([L, C_TOTAL], FP32, tag="consts")
            ident_sb = persist.tile([L, L], FP32, tag="identb")
            m_sb = persist.tile([L, NS, L], FP32, tag="m")
            m2_sb = persist.tile([L, NS, L], FP32, tag="m2")
            gat = persist.tile([L, NS, ROW], FP32, tag="gat")
            s_all = persist.tile([L, 2 * NS], FP32, tag="s_all")
            mx64 = persist.tile([L, 8 * NS], FP32, tag="mx64")
            ix64 = persist.tile([L, 8 * NS], U32, tag="ix64")
            nh8 = persist.tile([L, NS], FP32, tag="nh8")
            spraw8 = persist.tile([L, NS], FP32, tag="spraw8")
            m2at8 = persist.tile([L, NS], FP32, tag="m2at8")

            # ids first: the gathers depend only on it; m/m2 next so their
            # transfers finish before the gather stream needs the bandwidth
            nc.sync.dma_start(ids_sb[:], ids[:])
            nc.sync.dma_start(m_sb[:], m_in[:])
            nc.sync.dma_start(m2_sb[:], m2_in[:])
            nc.sync.dma_start(consts_sb[:], consts[:])
            nc.sync.dma_start(ident_sb[:], identb[:])

            # one indirect gather per sentence: each descriptor pulls the
            # interleaved V|W row (2112B) for one token ([P,1] offsets are the
            # only HW-supported form)
            for j in range(NS):
                nc.gpsimd.indirect_dma_start(
                    out=gat[:, j, :],
                    out_offset=None,
                    in_=table[:],
                    in_offset=bass.IndirectOffsetOnAxis(
                        ap=ids_sb[:, j:j + 1], axis=0
                    ),
                )

            iota = consts_sb[:, C_IOTA:C_IOTA + 128]

            # per sentence pair: transposes -> energies -> extraction, emitted
            # interleaved so each pair's extraction overlaps the next pair's
            # PE work
            for g in range(NS // 2):
                tq_sb = []
                for jj in range(2):
                    j = 2 * g + jj
                    # 4 PE transposes into one PSUM bank laid out
                    # [Vc0|Wc0|Vc1|Wc1], 2 half ACT copies so the chunk-0
                    # energy matmul can start while chunk 1 still transposes
                    tq = tq_psum.tile([L, 512], FP32, tag="tq_ps")
                    t = work.tile([L, 512], FP32, tag="tq_sb")
                    for k, off in enumerate(
                            (OFF_V, OFF_W, OFF_V + 128, OFF_W + 128)):
                        nc.tensor.matmul(
                            out=tq[:, k * 128:(k + 1) * 128],
                            lhsT=gat[:, j, off:off + 128],
                            rhs=ident_sb[:],
                            is_transpose=True,
                            start=(k == 0), stop=(k == 1),
                            skip_group_check=(k >= 2),
                        )
                        if k == 1:
                            nc.scalar.copy(out=t[:, 0:256], in_=tq[:, 0:256])
                    nc.scalar.copy(out=t[:, 256:512], in_=tq[:, 256:512])
                    tq_sb.append(t)

                # energies: the sentence pair shares one PSUM bank [128, 256]
                e2 = e_psum.tile([L, 256], FP32, tag="e2")
                for jj in range(2):
                    t = tq_sb[jj]
                    nc.tensor.matmul(out=e2[:, jj * 128:(jj + 1) * 128],
                                     lhsT=t[:, 128:256], rhs=t[:, 0:128],
                                     start=(jj == 0), stop=False)
                    nc.tensor.matmul(out=e2[:, jj * 128:(jj + 1) * 128],
                                     lhsT=t[:, 384:512], rhs=t[:, 256:384],
                                     start=False, stop=(jj == 1))

                # logits for the pair in one op
                logits = work.tile([L, 256], FP32, tag="logits")
                nc.vector.tensor_add(out=logits[:], in0=e2[:],
                                     in1=m_sb[:, 2 * g:2 * g + 2, :])

                for jj in range(2):
                    j = 2 * g + jj
                    lj = logits[:, jj * 128:(jj + 1) * 128]
                    nc.vector.max(out=mx64[:, 8 * j:8 * j + 8], in_=lj)
                    nc.vector.max_index(out=ix64[:, 8 * j:8 * j + 8],
                                        in_max=mx64[:, 8 * j:8 * j + 8],
                                        in_values=lj)
                    # pos score: E[l, hp[l]] extracted from PSUM
                    junk = work.tile([L, L], FP32, tag="junk")
                    nc.vector.scalar_tensor_tensor(
                        out=junk[:], in0=iota,
                        scalar=consts_sb[:, C_HP + j:C_HP + j + 1],
                        in1=e2[:, jj * 128:(jj + 1) * 128],
                        op0=mybir.AluOpType.is_equal,
                        op1=mybir.AluOpType.mult,
                        accum_out=spraw8[:, j:j + 1],
                    )
                # per-pair: neg-head indices -> f32, then extract M2[l, nh[l]]
                nc.vector.tensor_copy(out=nh8[:, 2 * g:2 * g + 2],
                                      in_=ix64[:, 16 * g:16 * g + 16:8])
                for jj in range(2):
                    j = 2 * g + jj
                    junk2 = work.tile([L, L], FP32, tag="junk2")
                    nc.vector.scalar_tensor_tensor(
                        out=junk2[:], in0=iota, scalar=nh8[:, j:j + 1],
                        in1=m2_sb[:, j, :],
                        op0=mybir.AluOpType.is_equal, op1=mybir.AluOpType.mult,
                        accum_out=m2at8[:, j:j + 1],
                    )

            # batched finalize
            wb8 = gat[:, :, OFF_WB]                      # [128, 8] strided
            mvr8 = consts_sb[:, C_MVR:C_MVR + NS]
            vbhp8 = consts_sb[:, C_VBHP:C_VBHP + NS]
            t1 = persist.tile([L, NS], FP32, tag="t1")
            nc.vector.tensor_add(out=t1[:], in0=spraw8[:], in1=vbhp8)
            t2 = persist.tile([L, NS], FP32, tag="t2")
            nc.vector.tensor_add(out=t2[:], in0=t1[:], in1=wb8)
            nc.vector.tensor_mul(out=s_all[:, 0:NS], in0=t2[:], in1=mvr8)

            t3 = persist.tile([L, NS], FP32, tag="t3")
            nc.vector.tensor_tensor(out=t3[:], in0=mx64[:, 0::8], in1=m2at8[:],
                                    op=mybir.AluOpType.subtract)
            t4 = persist.tile([L, NS], FP32, tag="t4")
            nc.vector.tensor_add(out=t4[:], in0=t3[:], in1=wb8)
            nc.vector.tensor_mul(out=s_all[:, NS:2 * NS], in0=t4[:], in1=mvr8)

            # reduce over positions: out[k] = sum_l s_all[l, k]
            s_ps = s_psum.tile([2 * NS, 1], FP32, tag="s_ps")
            nc.tensor.matmul(
                out=s_ps[:], lhsT=s_all[:],
                rhs=consts_sb[:, C_ONES_COL:C_ONES_COL + 1],
                start=True, stop=True,
            )
            s_out = persist.tile([2 * NS, 1], FP32, tag="s_out")
            nc.vector.tensor_copy(out=s_out[:], in_=s_ps[:])
            nc.sync.dma_start(out[:], s_out[:])

    nc.compile()
    return nc


_NC = None
TRACE = False
LAST_RES = None


def _get_nc():
    global _NC
    if _NC is None:
        _NC = build_nc()
    return _NC


def _gumbel_noise(words):
    """Gumbel noise exactly as jax.random.categorical(key(123), logits) draws it.

    jax.random bits differ between the cpu backend and the neuron/axon
    backend, so detect which backend generated the inputs (setup_inputs uses
    jax.random too) and draw the noise on that same backend.
    """
    import jax
    import jax.numpy as jnp

    def draw_words():
        ks = jax.random.split(jax.random.key(0), 8)
        return np.asarray(jax.device_get(
            jax.random.randint(ks[0], (B, L), 0, VOCAB, dtype=jnp.int32)))

    def draw_g():
        return np.asarray(jax.device_get(
            jax.random.gumbel(jax.random.key(123), (B, L, L), jnp.float32)))

    words = np.asarray(words)
    candidates = []
    try:
        cpu = jax.devices("cpu")[0]
        candidates.append(("cpu", cpu))
    except Exception:
        pass
    candidates.append(("default", None))

    for name, dev in candidates:
        try:
            if dev is not None:
                with jax.default_device(dev):
                    if np.array_equal(draw_words(), words):
                        return draw_g()
            else:
                if np.array_equal(draw_words(), words):
                    return draw_g()
        except Exception:
            continue
    # no backend reproduced the inputs; fall back to the default backend
    return draw_g()


def _prep_host(positive_sentences, mask, V, W, vb, wb):
    words = np.asarray(positive_sentences)[:, 0, :].astype(np.int64)
    heads = np.asarray(positive_sentences)[:, 1, :].astype(np.int64)
    pad = np.asarray(mask).astype(bool)
    vb = np.asarray(vb, dtype=np.float32)

    wordsp = np.where(pad, 0, words)                       # (B, L)
    mir = pad.copy()
    mir[:, 0] = True                                       # mask incl root
    hp = np.where(mir, -1.0, heads).astype(np.float32)
    mvr = (~mir).astype(np.float32)                        # (B, L)
    # vb at the positive heads (host-known indices)
    vbhp = np.where(mir, 0.0, vb[np.take_along_axis(wordsp, heads, axis=1)])
    vbhp = vbhp.astype(np.float32)

    # interleaved table: V(256) | vb | pad | W(256) | wb | pad per token
    tbl = np.zeros((VOCAB, ROW), dtype=np.float32)
    tbl[:, OFF_V:OFF_V + D] = np.asarray(V, dtype=np.float32)
    tbl[:, OFF_VB] = vb
    tbl[:, OFF_W:OFF_W + D] = np.asarray(W, dtype=np.float32)
    tbl[:, OFF_WB] = np.asarray(wb, dtype=np.float32)

    # logits additive term: gumbel noise + ok-masking; M2 = M - vb[words'[m]]
    G = _gumbel_noise(words)                               # (B, L, L)
    valid = ~pad
    eye = np.eye(L, dtype=bool)
    ok = valid[:, :, None] & valid[:, None, :] & ~eye[None, :, :]
    M = np.where(ok, G, G - BIG).astype(np.float32)        # (B, L, L)
    M2 = (M - vb[wordsp][:, None, :]).astype(np.float32)

    return wordsp, hp, mvr, vbhp, tbl, M, M2


def _build_in_maps(positive_sentences, mask, V, W, vb, wb):
    wordsp, hp, mvr, vbhp, tbl, M, M2 = _prep_host(
        positive_sentences, mask, V, W, vb, wb)

    identb = np.eye(L, dtype=np.float32)

    in_maps = []
    for c in range(N_CORES):
        s0 = c * SENT_PER_CORE
        ids = np.zeros((L, SENT_PER_CORE), dtype=np.int32)
        consts = np.zeros((L, C_TOTAL), dtype=np.float32)
        consts[:, C_IOTA:C_IOTA + 128] = np.arange(L, dtype=np.float32)[None, :]
        consts[:, C_ONES_COL] = 1.0
        for j in range(SENT_PER_CORE):
            b = s0 + j
            ids[:, j] = wordsp[b]
            consts[:, C_HP + j] = hp[b]
            consts[:, C_MVR + j] = mvr[b]
            consts[:, C_VBHP + j] = vbhp[b]
        m_core = np.ascontiguousarray(
            np.transpose(M[s0:s0 + SENT_PER_CORE], (1, 0, 2)))
        m2_core = np.ascontiguousarray(
            np.transpose(M2[s0:s0 + SENT_PER_CORE], (1, 0, 2)))
        in_maps.append({"table": tbl, "ids": ids, "m": m_core, "m2": m2_core,
                        "consts": consts, "identb": identb})
    return in_maps


def kernel(batch_id, positive_sentences, mask, V, W, vb, wb):
    in_maps = _build_in_maps(positive_sentences, mask, V, W, vb, wb)
    nc = _get_nc()
    res = bass_utils.run_bass_kernel_spmd(
        nc, in_maps, core_ids=list(range(N_CORES)), trace=TRACE
    )
    global LAST_RES
    LAST_RES = res

    pos = np.zeros(B, dtype=np.float32)
    neg = np.zeros(B, dtype=np.float32)
    for c in range(N_CORES):
        o = np.asarray(res.results[c]["out"]).reshape(2 * SENT_PER_CORE)
        pos[c * SENT_PER_CORE:(c + 1) * SENT_PER_CORE] = o[:SENT_PER_CORE]
        neg[c * SENT_PER_CORE:(c + 1) * SENT_PER_CORE] = o[SENT_PER_CORE:]
    return pos, neg
